# revision 16
# baseline (speedup 1.0000x reference)
# GAT layer kernel for Trainium2 (Bass/Tile), 8 NeuronCores data-parallel.
#
# Problem: B=16, S=64 -> 1024 independent 256-node graphs, F=O=64, H=1.
#   h = x @ W; a_s = h@att_src; a_d = h@att_dst
#   e[i,j] = leaky_relu(a_d[i] + a_s[j], 0.2) masked to (adj[j,i]!=0 | i==j)
#   alpha = softmax_j(e); out = alpha @ h + bias
#
# Layout on device: everything is computed in "source-major" [j, i] tiles
# (partition = source node j, free = target node i).
#
# v5 pipeline (per quad of 4 graphs = 2 pairs):
#   PE : h mms; a_s broadcast mms (a_s[j] = sum_f xT[f,j] vs[f]: the
#        h-matmul's xt-chunk stationary against a stride-0 vs-column rhs
#        writes a_s[j] into every score column); a_d broadcast mms.
#        The scores PSUM tile holds the complete z' = a_s[j] + a_d[i].
#        NO mask matmul: masking happens in the custom DVE op below.
#   ACT: u = Exp(z') straight from PSUM -> SBUF f16 (exact), one
#        instruction per pair; plus the quad h-copy (PSUM->SBUF f16).
#   DVE: one custom op per pair computes, in the f16 bits domain,
#          p_bits = select(adj_edge, max(U, 0.2*U + C1), 0)
#        i.e. exact exp for z'>=0, exp(0.2 z')*(1+-3.4%) for z'<0 (the
#        sub-1 weights), and an exact zero for masked edges - the
#        adjacency streams in as the second operand (fp8 {-1,0}).
#        Then reciprocal of the denominator + normalize (f16 out).
#   Aggregation matmuls consume p (bitcast f16) as the stationary with a
#   fused ones-column rhs giving the softmax denominator for free.
#   Output DMA'd as f16 on the sync queue; host upcasts to f32.

import os
import numpy as np

B, S, N, F, O = 16, 64, 256, 64, 64
G = B * S                  # 1024 graphs
NCORES = 8
GPC = G // NCORES          # 128 graphs per core
NEG_SLOPE = 0.2

# bits-domain pow-0.2 constant: v_bits = 0.2*U + EXP_C1 approximates
# bits(exp(0.2 z')) given U = bits(exp(z')); calibrated to 3.39% max rel
# err over z' in [-6.8, 0] (z' >= 0 uses U itself, exact).
EXP_C1 = 12250.0

BUFS_DEEP = 4
BUFS_MID = 3

_CACHE = {}
_OP = None


def _register_op():
    """Register the fused mask+lrelu-exp custom DVE op (runtime equivalent
    of adding it to concourse/dve_ops.py; the uop table is baked into the
    NEFF at compile time)."""
    global _OP
    if _OP is not None:
        return _OP
    import concourse.dve_ops as dve_ops
    from concourse.dve_spec import Spec, Src0, Src1, C0, C1, Zero, maxx, select, eq
    from concourse.dve_ops import DveOp
    from concourse.dve_table_gen import dve_ver_for

    name = "GAT_EXP_MASK_ANT"
    if name in dve_ops._SUB_OPCODE_FOR_NAME:
        _OP = next(op for op in dve_ops.OPS if op.name == name)
        return _OP
    spec = Spec(
        body=select(eq(Src1, Zero), maxx(Src0, Src0 * C0 + C1), Zero),
        reference=lambda in0, in1, s0, s1: np.where(
            in1 == 0, np.maximum(in0, in0 * s0 + s1), 0.0),
    )
    op = DveOp(name, spec, subdim=False, uops_sha={})
    dve_ops.OPS.append(op)
    dve_ops.CUSTOM_DVE_SPECS[name] = spec
    dve_ops._SUB_OPCODE_FOR_NAME[name] = (
        dve_ops._CUSTOM_DVE_ROW_BASE + len(dve_ops.OPS) - 1)
    ver = dve_ver_for("TRN2")
    try:
        op.compile(ver)
    except ValueError as e:
        import re
        m = re.search(r'uops_sha\["(\w+)"\]="([0-9a-f]+)"', str(e))
        op.uops_sha[m.group(1)] = m.group(2)
    op.compile(ver)
    _OP = op
    return op


def _build(with_bias, reps=1):
    import concourse.bass as bass
    import concourse.tile as tile
    import concourse.bacc as bacc
    import concourse.mybir as mybir

    op = _register_op()

    dt = mybir.dt
    f32, f16, i16 = dt.float32, dt.float16, dt.int16
    f8 = dt.float8e5
    AF = mybir.ActivationFunctionType
    ALU = mybir.AluOpType

    nc = bacc.Bacc("TRN2", debug=False)

    xT_d = nc.dram_tensor("xt", [GPC // 4, 128, 512], f16,
                          kind="ExternalInput").ap()
    adj_d = nc.dram_tensor("adjm", [GPC, N, N], f8, kind="ExternalInput").ap()
    wb_d = nc.dram_tensor("wb", [128, 64], f16, kind="ExternalInput").ap()
    vsb_d = nc.dram_tensor("vsb", [128, 256], f16, kind="ExternalInput").ap()
    vdb_d = nc.dram_tensor("vdb", [128, 128], f16, kind="ExternalInput").ap()
    if with_bias:
        bias_d = nc.dram_tensor("biasv", [O], f32, kind="ExternalInput").ap()
    out_d = nc.dram_tensor("out", [GPC, N, O], f16, kind="ExternalOutput").ap()

    with tile.TileContext(nc) as tc:
        from contextlib import ExitStack
        ctx = ExitStack()
        with ctx:
            consts = ctx.enter_context(tc.tile_pool(name="consts", bufs=1))
            xt_pool = ctx.enter_context(tc.tile_pool(name="xt", bufs=BUFS_DEEP))
            adj_pool = ctx.enter_context(tc.tile_pool(name="adj", bufs=BUFS_DEEP))
            h_pool = ctx.enter_context(tc.tile_pool(name="h", bufs=BUFS_DEEP))
            e_pool = ctx.enter_context(tc.tile_pool(name="e", bufs=BUFS_MID))
            p_pool = ctx.enter_context(tc.tile_pool(name="p", bufs=BUFS_MID))
            o_pool = ctx.enter_context(tc.tile_pool(name="o", bufs=BUFS_DEEP))
            ps_eb = ctx.enter_context(tc.tile_pool(name="ps_eb", bufs=2,
                                                   space="PSUM"))
            ps_h = ctx.enter_context(tc.tile_pool(name="ps_h", bufs=1, space="PSUM"))
            ps_ag = ctx.enter_context(tc.tile_pool(name="ps_ag", bufs=2, space="PSUM"))

            wb = consts.tile([128, 64], f16)
            nc.sync.dma_start(out=wb, in_=wb_d)
            vsb = consts.tile([128, 256], f16)
            nc.sync.dma_start(out=vsb, in_=vsb_d)
            vdb = consts.tile([128, 128], f16)
            nc.sync.dma_start(out=vdb, in_=vdb_d)
            if with_bias:
                bias_sb = consts.tile([128, O], f32)
                bias_b = bass.AP(
                    tensor=bias_d.tensor, offset=bias_d.offset,
                    ap=[[0, 128]] + list(bias_d.ap),
                )
                nc.sync.dma_start(out=bias_sb, in_=bias_b)

            # h_sb tiles carry a fused ones column per 65-wide block; the
            # h-copy only writes cols 0:64, so pre-bake the ones into every
            # pool rotation once, outside the loop.
            for _ in range(BUFS_DEEP):
                t = h_pool.tile([128, 520], f16, tag="h")
                tr = t.rearrange("p (b c) -> p b c", b=8)
                nc.vector.memset(tr[:, :, 64:65], 1.0)

            def body(_iv=None):
                n_quads = GPC // 4
                for q in range(n_quads):
                    emit_quad(q)

            def emit_quad(q):
                # ---- load 4 graphs' xT: parts 0:64 = g0,g1; 64:128 = g2,g3
                xt = xt_pool.tile([128, 512], f16)
                nc.sync.dma_start(out=xt, in_=xT_d[q])
                # ---- adjacency {-1 = no edge, 0 = edge} [j=128, (g, cj, i)]
                adjq = adj_pool.tile([128, 2048], f8)
                nc.sync.dma_start(
                    out=adjq,
                    in_=adj_d[4 * q: 4 * q + 4].rearrange(
                        "g (cj p) i -> p (g cj) i", cj=2
                    ),
                )
                outq = o_pool.tile([128, 512], f16, tag="out")

                # ---- h matmuls + score matmuls (z' = a_s[j] + a_d[i])
                psh = ps_h.tile([128, 1024], f32)   # pair pr at offset 512*pr
                ebs = []
                for pr in range(2):
                    lo = 64 * pr
                    eb = ps_eb.tile([128, 1024], f32, name="eb")
                    ebs.append(eb)
                    for gl in range(2):
                        xs = xt[lo: lo + 64, 256 * gl: 256 * gl + 256]
                        xs2 = bass.AP(
                            tensor=xs.tensor, offset=xs.offset,
                            ap=[xs.ap[0], [0, 2]] + list(xs.ap[1:]),
                        )
                        nc.tensor.matmul(
                            out=eb[:, 512 * gl: 512 * gl + 512],
                            lhsT=vdb[lo: lo + 64],
                            rhs=xs2,
                            start=True, stop=False,
                        )
                    for b in range(4):
                        gl, c = b // 2, b % 2
                        chunk = xt[lo: lo + 64,
                                   256 * gl + 128 * c: 256 * gl + 128 * c + 128]
                        nc.tensor.matmul(
                            out=psh[:, 512 * pr + 64 * b: 512 * pr + 64 * b + 64],
                            lhsT=chunk, rhs=wb[lo: lo + 64],
                            start=True, stop=True,
                        )
                        # a_s[j] into the (gl, cj=c) score block: same
                        # stationary, 256 replicated vs columns as rhs
                        nc.tensor.matmul(
                            out=eb[:, 512 * gl + 256 * c: 512 * gl + 256 * c + 256],
                            lhsT=chunk, rhs=vsb[lo: lo + 64],
                            start=False, stop=True,
                        )

                # ---- h blocks -> SBUF f16 (ones cols pre-baked): ACT copies
                h_sb = h_pool.tile([128, 520], f16, tag="h")
                for pr in range(2):
                    psh_v = bass.AP(
                        tensor=psh.tensor, offset=psh.offset + 512 * pr,
                        ap=[psh.ap[0], [64, 4], [1, 64]],
                    )
                    h_v = bass.AP(
                        tensor=h_sb.tensor, offset=h_sb.offset + 260 * pr,
                        ap=[h_sb.ap[0], [65, 4], [1, 64]],
                    )
                    nc.scalar.copy(h_v, psh_v)

                # ---- u = exp(z') exact (ACT, straight from PSUM)
                # ---- p = select(edge, max(U, 0.2U + C1), 0)  (custom DVE)
                aggs = []
                for pr in range(2):
                    u = e_pool.tile([128, 1024], f16, tag="u")
                    nc.scalar.activation(out=u, in_=ebs[pr], func=AF.Exp)
                    p_i16 = p_pool.tile([128, 1024], i16, tag="p")
                    nc.vector._custom_dve(
                        op, out=p_i16, in0=u.bitcast(i16),
                        in1=adjq[:, 1024 * pr: 1024 * pr + 1024],
                        s0=NEG_SLOPE, s1=EXP_C1,
                    )
                    p_sb = p_i16.bitcast(f16)
                    agg = ps_ag.tile([128, 260], f32, name="agg")
                    aggs.append(agg)

                    # ---- aggregation + denominator: [out_unnorm | S]
                    for a in range(4):
                        gl, ci = a // 2, a % 2
                        for cj in range(2):
                            lhsT = p_sb[:, 512 * gl + 256 * cj + 128 * ci:
                                        512 * gl + 256 * cj + 128 * ci + 128]
                            rhs = h_sb[:, 65 * (4 * pr + 2 * gl + cj):
                                       65 * (4 * pr + 2 * gl + cj) + 65]
                            nc.tensor.matmul(
                                out=agg[:, 65 * a: 65 * a + 65],
                                lhsT=lhsT, rhs=rhs,
                                start=(cj == 0), stop=(cj == 1),
                            )

                # ---- normalize (per pair: reciprocal + tensor_tensor)
                for pr in range(2):
                    aggq = aggs[pr]
                    rs = o_pool.tile([128, 4], f32, tag="rs")
                    den_v = bass.AP(
                        tensor=aggq.tensor, offset=aggq.offset + 64,
                        ap=[aggq.ap[0], [65, 4], [1, 1]],
                    )
                    nc.vector.reciprocal(
                        out=rs.rearrange("p (a c) -> p a c", a=4),
                        in_=den_v,
                    )
                    agg_v = bass.AP(
                        tensor=aggq.tensor, offset=aggq.offset,
                        ap=[aggq.ap[0], [65, 4], [1, 64]],
                    )
                    rs_b = bass.AP(
                        tensor=rs.tensor, offset=rs.offset,
                        ap=[rs.ap[0], [1, 4], [0, 64]],
                    )
                    out_v = bass.AP(
                        tensor=outq.tensor, offset=outq.offset + 256 * pr,
                        ap=[outq.ap[0], [64, 4], [1, 64]],
                    )
                    nc.vector.tensor_tensor(
                        out=out_v, in0=agg_v, in1=rs_b, op=ALU.mult,
                    )
                    if with_bias:
                        bias_b4 = bass.AP(
                            tensor=bias_sb.tensor, offset=bias_sb.offset,
                            ap=[bias_sb.ap[0], [0, 4], [1, 64]],
                        )
                        nc.vector.tensor_tensor(
                            out=out_v, in0=out_v, in1=bias_b4, op=ALU.add,
                        )

                nc.sync.dma_start(
                    out=out_d[4 * q: 4 * q + 4].rearrange(
                        "g (ci p) o -> p (g ci) o", ci=2
                    ),
                    in_=outq,
                )

            if reps == 1:
                body()
            else:
                with tc.For_i(0, reps, 1) as _i:
                    body()
    nc.compile()
    return nc


def kernel(x, adj, W, att_src, att_dst, bias):
    from concourse.bass_utils import run_bass_kernel_spmd

    x = np.asarray(x, dtype=np.float32)
    adj = np.asarray(adj)
    W = np.asarray(W, dtype=np.float32)
    att_src = np.asarray(att_src, dtype=np.float32)
    att_dst = np.asarray(att_dst, dtype=np.float32)
    bias = np.asarray(bias, dtype=np.float32)

    # ---- host-side marshalling
    # per-quad SBUF image: [q, part=(gp, f), free=(gl, i)]
    xg = np.ascontiguousarray(
        x.reshape(G // 4, 2, 2, N, F)                    # [q, gp, gl, n, f]
        .transpose(0, 1, 4, 2, 3)                        # [q, gp, f, gl, n]
        .reshape(G // 4, 128, 512)).astype(np.float16)
    ar = np.arange(N)
    import ml_dtypes
    adjm = (adj.reshape(G, N, N) == 0).astype(np.int8)
    np.negative(adjm, out=adjm)                          # {-1 no edge, 0 edge}
    adjm[:, ar, ar] = 0                                  # self loops always kept
    adjm = adjm.astype(ml_dtypes.float8_e5m2)

    wb = np.zeros((128, 64), np.float16)
    wb[0:64] = W
    wb[64:128] = W
    vs = W @ att_src.reshape(-1)                         # [F]
    vsb = np.zeros((128, 256), np.float16)
    vsb[0:64] = np.repeat(vs[:, None], 256, axis=1)
    vsb[64:128] = vsb[0:64]
    vd = W @ att_dst.reshape(-1)                         # [F]
    vdb = np.zeros((128, 128), np.float16)
    vdb[0:64] = np.repeat(vd[:, None], 128, axis=1)
    vdb[64:128] = vdb[0:64]

    with_bias = bool(np.any(bias))
    key = ("gat", with_bias)
    if key not in _CACHE:
        _CACHE[key] = _build(with_bias)
    nc = _CACHE[key]

    qpc = GPC // 4
    in_maps = []
    for c in range(NCORES):
        m = {
            "xt": np.ascontiguousarray(xg[c * qpc:(c + 1) * qpc]),
            "adjm": np.ascontiguousarray(adjm[c * GPC:(c + 1) * GPC]),
            "wb": wb,
            "vsb": vsb,
            "vdb": vdb,
        }
        if with_bias:
            m["biasv"] = bias
        in_maps.append(m)

    trace = os.environ.get("GAT_TRACE", "0") == "1"
    res = run_bass_kernel_spmd(
        nc, in_maps, core_ids=list(range(NCORES)), trace=trace,
    )
    global LAST_EXEC_NS, _LAST_NC, _LAST_IN_MAPS
    LAST_EXEC_NS = res.exec_time_ns
    _LAST_NC = nc
    _LAST_IN_MAPS = in_maps

    out = np.concatenate([r["out"] for r in res.results], axis=0)
    return out.reshape(B, S, N, O).astype(np.float32)


LAST_EXEC_NS = None
